# revision 2
# baseline (speedup 1.0000x reference)
"""ChannelAttentionBlock3d kernel for 8 trn2 NeuronCores.

Math (per batch b, xf = x.reshape(B, C, N)):
    a1   = xf @ xf^T                      (C, C)
    aff  = a1 @ a1                        (C, C)
    P    = softmax(rowmax(aff) - aff)     == exp(rowmin(aff) - aff) / sum
    out  = gamma * (P @ xf) + xf          == (I + gamma*P) @ xf

Sharding: 8 cores = 4 batches x 2 N-halves (NH = 16384).
  - phase A: each core computes its N-half partial of a1 in bf16 hi/lo
    precision: a1 = hi@hi^T + hi@lo^T + (hi@lo^T)^T  (x = hi + lo exactly
    split into two bf16 words; the dropped lo@lo^T term only perturbs the
    affinity diagonal, which the softmax ignores).
  - phase B: AllReduce of the (256,256) fp32 partial within core pairs.
  - phase C: aff = a1@a1 in fp32 (tiny), softmax row-wise, Q = I + gamma*P,
    Q transposed on the PE.
  - phase D: out_half = Q @ (hi + lo) over the core's N-half, streamed out.

The host ships x pre-split in both layouts (hi/lo natural C-major for the
phase-D GEMM, hiT/loT N-major for the phase-A GEMM) because the two GEMMs
contract over different axes and on-device transposition is slower than DMA.
"""

import os
import sys

import numpy as np

for _p in ("/opt/trn_rl_repo",):
    if _p not in sys.path:
        sys.path.insert(0, _p)

import ml_dtypes

BF16 = ml_dtypes.bfloat16

B, C, N = 4, 256, 32 * 32 * 32
N_CORES = 8
NH = N // 2          # per-core slice of N
KB = 4               # 128-row k-tiles per DMA batch in phase A
CHUNK = 512          # phase-D output chunk width


def build_nc(nh=NH, n_cores=N_CORES):
    import concourse.bacc as bacc
    from concourse import mybir, tile

    f32 = mybir.dt.float32
    bf16 = mybir.dt.bfloat16
    AX = mybir.AxisListType
    OP = mybir.AluOpType
    ACT = mybir.ActivationFunctionType

    kt = nh // 128          # total 128-row k-tiles in phase A
    nb = kt // KB           # DMA batches in phase A
    nch = nh // CHUNK       # phase-D chunks per c-block

    nc = bacc.Bacc(
        "TRN2",
        target_bir_lowering=False,
        debug=False,
        enable_asserts=False,
        num_devices=n_cores,
    )

    hiT_d = nc.dram_tensor("hiT", [nh, C], bf16, kind="ExternalInput").ap()
    loT_d = nc.dram_tensor("loT", [nh, C], bf16, kind="ExternalInput").ap()
    hi_d = nc.dram_tensor("hi", [C, nh], bf16, kind="ExternalInput").ap()
    lo_d = nc.dram_tensor("lo", [C, nh], bf16, kind="ExternalInput").ap()
    eye_d = nc.dram_tensor("eye", [C, C], f32, kind="ExternalInput").ap()
    i128_d = nc.dram_tensor("i128", [128, 128], f32, kind="ExternalInput").ap()
    gcol_d = nc.dram_tensor("gcol", [128, 1], f32, kind="ExternalInput").ap()
    out_d = nc.dram_tensor("out", [C, nh], f32, kind="ExternalOutput").ap()

    with tile.TileContext(nc) as tc:
        with (
            tc.tile_pool(name="big", bufs=1) as big,
            tc.tile_pool(name="small", bufs=1) as small,
            tc.tile_pool(name="ktp", bufs=6) as ktp,
            tc.tile_pool(name="outp", bufs=6) as outp,
            tc.tile_pool(name="ps", bufs=4, space="PSUM") as ps,
            tc.tile_pool(name="psT", bufs=2, space="PSUM") as psT,
            tc.tile_pool(name="dram", bufs=1, space="DRAM") as dram,
        ):
            # resident natural-layout operands for phase D (c-major, 2 c-blocks)
            hi_s = big.tile([128, 2, nh], bf16)
            lo_s = big.tile([128, 2, nh], bf16)
            # aux constants
            eye_s = small.tile([128, 2, C], f32)
            i128_s = small.tile([128, 128], f32)
            gcol_s = small.tile([128, 1], f32)
            nc.sync.dma_start(i128_s[:], i128_d)
            nc.sync.dma_start(gcol_s[:], gcol_d)
            for j in range(2):
                nc.sync.dma_start(eye_s[:, j, :], eye_d[j * 128:(j + 1) * 128, :])

            # ---------------- phase A: a1 partial = hi@hi^T + cross ----------
            hiT_r = hiT_d.rearrange("(g t p) c -> g p t c", t=KB, p=128)
            loT_r = loT_d.rearrange("(g t p) c -> g p t c", t=KB, p=128)

            hh = [ps.tile([128, C], f32, name=f"hh{j}", tag="acc") for j in range(2)]
            hl = [ps.tile([128, C], f32, name=f"hl{j}", tag="acc") for j in range(2)]

            # natural-layout loads staggered through the phase-A loop so they
            # don't starve the k-tile stream of DMA bandwidth
            big_loads = []
            qn = nh // 4
            for src_d, dst_s in ((hi_d, hi_s), (lo_d, lo_s)):
                for j in range(2):
                    for q in range(4):
                        big_loads.append(
                            (dst_s[:, j, q * qn:(q + 1) * qn],
                             src_d[j * 128:(j + 1) * 128, q * qn:(q + 1) * qn])
                        )
            stride = max(1, nb // len(big_loads))

            for g in range(nb):
                th = ktp.tile([128, KB, C], bf16, tag="th")
                tl = ktp.tile([128, KB, C], bf16, tag="tl")
                nc.sync.dma_start(th[:], hiT_r[g])
                nc.sync.dma_start(tl[:], loT_r[g])
                for t in range(KB):
                    k = g * KB + t
                    for j in range(2):
                        lhs = th[:, t, j * 128:(j + 1) * 128]
                        nc.tensor.matmul(hh[j][:], lhs, th[:, t, :],
                                         start=(k == 0), stop=(k == kt - 1))
                        nc.tensor.matmul(hl[j][:], lhs, tl[:, t, :],
                                         start=(k == 0), stop=(k == kt - 1))
                while big_loads and g % stride == stride - 1:
                    dst, src = big_loads.pop(0)
                    nc.sync.dma_start(dst, src)
                    if g < nb - 1:
                        break
            for dst, src in big_loads:
                nc.sync.dma_start(dst, src)

            # a1 = hh + hl + hl^T
            s_hl = small.tile([128, 2, C], f32)
            a1h = small.tile([128, 2, C], f32)
            for j in range(2):
                nc.scalar.copy(s_hl[:, j, :], hl[j][:])
                nc.vector.tensor_tensor(a1h[:, j, :], hh[j][:], s_hl[:, j, :], op=OP.add)
            a1_s = small.tile([128, 2, C], f32)
            for j in range(2):
                for i in range(2):
                    tp = psT.tile([128, 128], f32, tag="tp")
                    nc.tensor.transpose(tp[:], s_hl[:, i, j * 128:(j + 1) * 128], i128_s[:])
                    nc.vector.tensor_tensor(
                        a1_s[:, j, i * 128:(i + 1) * 128],
                        a1h[:, j, i * 128:(i + 1) * 128], tp[:], op=OP.add)

            # ---------------- phase B: pair AllReduce of the partial ---------
            a1p_d = dram.tile([C, C], f32)
            ar_d = dram.tile([C, C], f32)
            for j in range(2):
                nc.sync.dma_start(a1p_d[j * 128:(j + 1) * 128, :], a1_s[:, j, :])
            groups = [[2 * i, 2 * i + 1] for i in range(n_cores // 2)]
            if n_cores == 1:
                groups = [[0]]
            nc.gpsimd.collective_compute(
                "AllReduce", OP.add, replica_groups=groups,
                ins=[a1p_d.opt()], outs=[ar_d.opt()])
            a1f = small.tile([128, 2, C], f32)
            for j in range(2):
                nc.sync.dma_start(a1f[:, j, :], ar_d[j * 128:(j + 1) * 128, :])

            # ---------------- phase C: affinity, softmax, Q = I + g*P --------
            mj = small.tile([128, 2, 1], f32)
            sj = small.tile([128, 2, 1], f32)
            rj = small.tile([128, 2, 1], f32)
            rg = small.tile([128, 2, 1], f32)
            tj = small.tile([128, 2, C], f32)
            qf = small.tile([128, 2, C], f32)
            qt = small.tile([128, 2, C], bf16)
            for j in range(2):
                af = ps.tile([128, C], f32, name=f"af{j}", tag="acc")
                for k in range(2):
                    # a1 is symmetric, so a1 block (k,j) serves as lhsT
                    nc.tensor.matmul(af[:], a1f[:, k, j * 128:(j + 1) * 128],
                                     a1f[:, k, :], start=(k == 0), stop=(k == 1))
                nc.vector.tensor_reduce(mj[:, j, :], af[:], axis=AX.X, op=OP.min)
                nc.scalar.activation(tj[:, j, :], af[:], ACT.Exp,
                                     bias=mj[:, j, :], scale=-1.0,
                                     accum_out=sj[:, j, :])
                nc.vector.reciprocal(rj[:, j, :], sj[:, j, :])
                nc.vector.tensor_tensor(rg[:, j, :], rj[:, j, :], gcol_s[:], op=OP.mult)
                nc.vector.tensor_scalar(qf[:, j, :], tj[:, j, :], rg[:, j, :],
                                        None, op0=OP.mult)
                nc.vector.tensor_tensor(qf[:, j, :], qf[:, j, :], eye_s[:, j, :],
                                        op=OP.add)
            for k in range(2):
                for j in range(2):
                    tp = psT.tile([128, 128], f32, tag="tp")
                    nc.tensor.transpose(tp[:], qf[:, j, k * 128:(k + 1) * 128], i128_s[:])
                    nc.scalar.copy(qt[:, k, j * 128:(j + 1) * 128], tp[:])

            # ---------------- phase D: out = Q @ (hi + lo) -------------------
            for j in range(2):
                jsl = slice(j * 128, (j + 1) * 128)
                for ch in range(nch):
                    w = ps.tile([128, CHUNK], f32, tag="acc")
                    csl = slice(ch * CHUNK, (ch + 1) * CHUNK)
                    nc.tensor.matmul(w[:], qt[:, 0, jsl], hi_s[:, 0, csl],
                                     start=True, stop=False)
                    nc.tensor.matmul(w[:], qt[:, 0, jsl], lo_s[:, 0, csl],
                                     start=False, stop=False)
                    nc.tensor.matmul(w[:], qt[:, 1, jsl], hi_s[:, 1, csl],
                                     start=False, stop=False)
                    nc.tensor.matmul(w[:], qt[:, 1, jsl], lo_s[:, 1, csl],
                                     start=False, stop=True)
                    o = outp.tile([128, CHUNK], f32, tag="o")
                    if ch % 2 == 0:
                        nc.scalar.copy(o[:], w[:])
                    else:
                        nc.vector.tensor_copy(o[:], w[:])
                    nc.sync.dma_start(out_d[jsl, csl], o[:])

    nc.compile()
    return nc


_NC_CACHE = {}


def _get_nc(nh=NH, n_cores=N_CORES):
    key = (nh, n_cores)
    if key not in _NC_CACHE:
        _NC_CACHE[key] = build_nc(nh, n_cores)
    return _NC_CACHE[key]


def make_in_maps(x, gamma, nh=NH, n_cores=N_CORES):
    xf = np.ascontiguousarray(x.reshape(B, C, N).astype(np.float32))
    hi = xf.astype(BF16)
    lo = (xf - hi.astype(np.float32)).astype(BF16)
    eye = np.eye(C, dtype=np.float32)
    i128 = np.eye(128, dtype=np.float32)
    gcol = np.full((128, 1), float(np.asarray(gamma).reshape(-1)[0]), np.float32)

    in_maps = []
    for c in range(n_cores):
        b, h = c // 2, c % 2
        sl = slice(h * nh, (h + 1) * nh)
        hi_n = np.ascontiguousarray(hi[b, :, sl])
        lo_n = np.ascontiguousarray(lo[b, :, sl])
        in_maps.append({
            "hi": hi_n,
            "lo": lo_n,
            "hiT": np.ascontiguousarray(hi_n.T),
            "loT": np.ascontiguousarray(lo_n.T),
            "eye": eye,
            "i128": i128,
            "gcol": gcol,
        })
    return in_maps


def kernel(x, gamma):
    from concourse import bass_utils

    nc = _get_nc()
    in_maps = make_in_maps(x, gamma)
    res = bass_utils.run_bass_kernel_spmd(nc, in_maps, core_ids=list(range(N_CORES)))
    out = np.empty((B, C, N), np.float32)
    for c in range(N_CORES):
        b, h = c // 2, c % 2
        out[b, :, h * NH:(h + 1) * NH] = res.results[c]["out"]
    return out.reshape(x.shape).astype(x.dtype)


# revision 5
# speedup vs baseline: 777.8561x; 777.8561x over previous
"""ChannelAttentionBlock3d kernel for 8 trn2 NeuronCores.

Math (per batch b, xf = x.reshape(B, C, N)):
    a1   = xf @ xf^T                      (C, C)
    aff  = a1 @ a1                        (C, C)
    P    = softmax(rowmax(aff) - aff)     == exp(rowmin(aff) - aff) / sum
    out  = gamma * (P @ xf) + xf          == (I + gamma*P) @ xf

Sharding: 8 cores = 4 batches x 2 N-halves (NH = 16384).
  - phase A: each core computes its N-half partial of a1 in bf16 hi/lo
    precision: a1 = hi@hi^T + hi@lo^T + (hi@lo^T)^T  (x = hi + lo split into
    two bf16 words; the dropped lo@lo^T term only perturbs the affinity
    diagonal, which the softmax ignores). hi and lo rows are interleaved
    host-side into one (NH, 512) array so a single 512-wide moving operand
    produces both the hi@hi^T and hi@lo^T psum columns per matmul.
  - phase B: AllGather of the (256,256) fp32 partial within core pairs,
    summed locally (bit-identical on both cores of a pair).
  - phase C: aff = a1@a1 in fp32 (tiny), softmax row-wise, Q = I + gamma*P,
    Q transposed on the PE.
  - phase D: out_half = Q @ x16 over the core's N-half in fp16 (x error
    ~2^-11, far below the tolerance), streamed out as fp16 and upcast on
    the host.

The host ships x pre-split in both layouts (fp16 natural C-major for the
phase-D GEMM, bf16-pair N-major for the phase-A GEMM) because the two GEMMs
contract over different axes and on-device transposition is slower than DMA.
"""

import os
import sys

import numpy as np

for _p in ("/opt/trn_rl_repo",):
    if _p not in sys.path:
        sys.path.insert(0, _p)

import ml_dtypes

BF16 = ml_dtypes.bfloat16

B, C, N = 4, 256, 32 * 32 * 32
N_CORES = 8
NH = N // 2          # per-core slice of N
KB = 4               # 128-row k-tiles per DMA batch in phase A
CHUNK = 512          # phase-D output chunk width


def build_nc(nh=NH, n_cores=N_CORES, reps=1, use_cc=True):
    import concourse.bacc as bacc
    from concourse import mybir, tile

    f32 = mybir.dt.float32
    f16 = mybir.dt.float16
    bf16 = mybir.dt.bfloat16
    AX = mybir.AxisListType
    OP = mybir.AluOpType
    ACT = mybir.ActivationFunctionType

    kt = nh // 128          # total 128-row k-tiles in phase A
    nb = kt // KB           # DMA batches in phase A
    nch = nh // CHUNK       # phase-D chunks per c-block

    nc = bacc.Bacc(
        "TRN2",
        target_bir_lowering=False,
        debug=False,
        enable_asserts=False,
        num_devices=n_cores,
    )

    hlT_d = nc.dram_tensor("hlT", [nh, 2 * C], bf16, kind="ExternalInput").ap()
    x16_d = nc.dram_tensor("x16", [C, nh], f16, kind="ExternalInput").ap()
    eye_d = nc.dram_tensor("eye", [C, C], f32, kind="ExternalInput").ap()
    i128_d = nc.dram_tensor("i128", [128, 128], f32, kind="ExternalInput").ap()
    gcol_d = nc.dram_tensor("gcol", [128, 1], f32, kind="ExternalInput").ap()
    out_d = nc.dram_tensor("out", [C, nh], f16, kind="ExternalOutput").ap()

    with tile.TileContext(nc) as tc:
        with (
            tc.tile_pool(name="big", bufs=1) as big,
            tc.tile_pool(name="small", bufs=1) as small,
            tc.tile_pool(name="ktp", bufs=8) as ktp,
            tc.tile_pool(name="outp", bufs=6) as outp,
            tc.tile_pool(name="ps", bufs=4, space="PSUM") as ps,
            tc.tile_pool(name="psT", bufs=2, space="PSUM") as psT,
            tc.tile_pool(name="dram", bufs=1, space="DRAM") as dram,
        ):
            # resident natural-layout fp16 operand for phase D (2 c-blocks)
            x16_s = big.tile([128, 2, nh], f16)
            # aux constants
            eye_s = small.tile([128, 2, C], f32)
            i128_s = small.tile([128, 128], f32)
            gcol_s = small.tile([128, 1], f32)
            nc.sync.dma_start(i128_s[:], i128_d)
            nc.sync.dma_start(gcol_s[:], gcol_d)
            for j in range(2):
                nc.sync.dma_start(eye_s[:, j, :], eye_d[j * 128:(j + 1) * 128, :])

            hlT_r = hlT_d.rearrange("(g t p) c -> g p t c", t=KB, p=128)

            for rep in range(reps):
                # ------------- phase A: [a1_hh | a1_hl] = hiT.T @ [hiT|loT] --
                acc = [ps.tile([128, 2 * C], f32, name=f"acc{j}", tag="acc")
                       for j in range(2)]

                # natural-layout fp16 loads staggered through the phase-A loop
                big_loads = []
                qn = nh // 4
                for j in range(2):
                    for q in range(4):
                        big_loads.append(
                            (x16_s[:, j, q * qn:(q + 1) * qn],
                             x16_d[j * 128:(j + 1) * 128, q * qn:(q + 1) * qn])
                        )
                stride = max(1, nb // (len(big_loads) + 1))

                for g in range(nb):
                    th = ktp.tile([128, KB, 2 * C], bf16, tag="th")
                    nc.sync.dma_start(th[:], hlT_r[g])
                    for t in range(KB):
                        k = g * KB + t
                        for j in range(2):
                            nc.tensor.matmul(acc[j][:],
                                             th[:, t, j * 128:(j + 1) * 128],
                                             th[:, t, :],
                                             start=(k == 0), stop=(k == kt - 1))
                    if big_loads and g % stride == stride - 1:
                        dst, src = big_loads.pop(0)
                        nc.sync.dma_start(dst, src)
                for dst, src in big_loads:
                    nc.sync.dma_start(dst, src)

                # a1 = hh + hl + hl^T
                s_hl = small.tile([128, 2, C], f32)
                a1h = small.tile([128, 2, C], f32)
                for j in range(2):
                    nc.scalar.copy(s_hl[:, j, :], acc[j][:, C:2 * C])
                    nc.vector.tensor_tensor(a1h[:, j, :], acc[j][:, 0:C],
                                            s_hl[:, j, :], op=OP.add)
                a1_s = small.tile([128, 2, C], f32)
                for j in range(2):
                    for i in range(2):
                        tp = psT.tile([128, 128], f32, tag="tp")
                        nc.tensor.transpose(tp[:], s_hl[:, i, j * 128:(j + 1) * 128],
                                            i128_s[:])
                        nc.vector.tensor_tensor(
                            a1_s[:, j, i * 128:(i + 1) * 128],
                            a1h[:, j, i * 128:(i + 1) * 128], tp[:], op=OP.add)

                # ------------- phase B: pair AllGather + local sum -----------
                a1f = small.tile([128, 2, C], f32)
                if use_cc and n_cores > 1:
                    a1p_d = dram.tile([C, C], f32)
                    ag_d = dram.tile([2 * C, C], f32)
                    for j in range(2):
                        nc.sync.dma_start(a1p_d[j * 128:(j + 1) * 128, :],
                                          a1_s[:, j, :])
                    groups = [[2 * i, 2 * i + 1] for i in range(n_cores // 2)]
                    nc.gpsimd.collective_compute(
                        "AllGather", OP.bypass, replica_groups=groups,
                        ins=[a1p_d.opt()], outs=[ag_d.opt()])
                    ag0 = small.tile([128, 2, C], f32)
                    ag1 = small.tile([128, 2, C], f32)
                    for j in range(2):
                        nc.sync.dma_start(ag0[:, j, :],
                                          ag_d[j * 128:(j + 1) * 128, :])
                        nc.sync.dma_start(ag1[:, j, :],
                                          ag_d[C + j * 128:C + (j + 1) * 128, :])
                    for j in range(2):
                        nc.vector.tensor_tensor(a1f[:, j, :], ag0[:, j, :],
                                                ag1[:, j, :], op=OP.add)
                else:
                    for j in range(2):
                        nc.vector.tensor_copy(a1f[:, j, :], a1_s[:, j, :])

                # ------------- phase C: affinity, softmax, Q = I + g*P -------
                mj = small.tile([128, 2, 1], f32)
                sj = small.tile([128, 2, 1], f32)
                rj = small.tile([128, 2, 1], f32)
                rg = small.tile([128, 2, 1], f32)
                tj = small.tile([128, 2, C], f32)
                qf = small.tile([128, 2, C], f32)
                qt = small.tile([128, 2, C], f16)
                for j in range(2):
                    af = ps.tile([128, C], f32, name=f"af{j}", tag="acc")
                    for k in range(2):
                        # a1 is symmetric, so a1 block (k,j) serves as lhsT
                        nc.tensor.matmul(af[:], a1f[:, k, j * 128:(j + 1) * 128],
                                         a1f[:, k, :], start=(k == 0), stop=(k == 1))
                    nc.vector.tensor_reduce(mj[:, j, :], af[:], axis=AX.X, op=OP.min)
                    nc.scalar.activation(tj[:, j, :], af[:], ACT.Exp,
                                         bias=mj[:, j, :], scale=-1.0,
                                         accum_out=sj[:, j, :])
                    nc.vector.reciprocal(rj[:, j, :], sj[:, j, :])
                    nc.vector.tensor_tensor(rg[:, j, :], rj[:, j, :], gcol_s[:],
                                            op=OP.mult)
                    nc.vector.tensor_scalar(qf[:, j, :], tj[:, j, :], rg[:, j, :],
                                            None, op0=OP.mult)
                    nc.vector.tensor_tensor(qf[:, j, :], qf[:, j, :],
                                            eye_s[:, j, :], op=OP.add)
                for k in range(2):
                    for j in range(2):
                        tp = psT.tile([128, 128], f32, tag="tp")
                        nc.tensor.transpose(tp[:], qf[:, j, k * 128:(k + 1) * 128],
                                            i128_s[:])
                        nc.scalar.copy(qt[:, k, j * 128:(j + 1) * 128], tp[:])

                # ------------- phase D: out = Q @ x16 ------------------------
                for j in range(2):
                    jsl = slice(j * 128, (j + 1) * 128)
                    for ch in range(nch):
                        w = ps.tile([128, CHUNK], f32, tag="acc")
                        csl = slice(ch * CHUNK, (ch + 1) * CHUNK)
                        nc.tensor.matmul(w[:], qt[:, 0, jsl], x16_s[:, 0, csl],
                                         start=True, stop=False)
                        nc.tensor.matmul(w[:], qt[:, 1, jsl], x16_s[:, 1, csl],
                                         start=False, stop=True)
                        o = outp.tile([128, CHUNK], f16, tag="o")
                        if ch % 2 == 0:
                            nc.scalar.copy(o[:], w[:])
                        else:
                            nc.vector.tensor_copy(o[:], w[:])
                        nc.sync.dma_start(out_d[jsl, csl], o[:])

    nc.compile()
    return nc


_NC_CACHE = {}


def _get_nc(nh=NH, n_cores=N_CORES):
    key = (nh, n_cores)
    if key not in _NC_CACHE:
        _NC_CACHE[key] = build_nc(nh, n_cores)
    return _NC_CACHE[key]


def make_in_maps(x, gamma, nh=NH, n_cores=N_CORES):
    xf = np.ascontiguousarray(x.reshape(B, C, N).astype(np.float32))
    hi = xf.astype(BF16)
    lo = (xf - hi.astype(np.float32)).astype(BF16)
    x16 = xf.astype(np.float16)
    eye = np.eye(C, dtype=np.float32)
    i128 = np.eye(128, dtype=np.float32)
    gcol = np.full((128, 1), float(np.asarray(gamma).reshape(-1)[0]), np.float32)

    in_maps = []
    for c in range(n_cores):
        b, h = c // 2, c % 2
        sl = slice(h * nh, (h + 1) * nh)
        hlT = np.empty((nh, 2 * C), BF16)
        hlT[:, :C] = hi[b, :, sl].T
        hlT[:, C:] = lo[b, :, sl].T
        in_maps.append({
            "hlT": hlT,
            "x16": np.ascontiguousarray(x16[b, :, sl]),
            "eye": eye,
            "i128": i128,
            "gcol": gcol,
        })
    return in_maps


def kernel(x, gamma):
    from concourse import bass_utils

    nc = _get_nc()
    in_maps = make_in_maps(x, gamma)
    res = bass_utils.run_bass_kernel_spmd(nc, in_maps, core_ids=list(range(N_CORES)))
    out = np.empty((B, C, N), np.float32)
    for c in range(N_CORES):
        b, h = c // 2, c % 2
        out[b, :, h * NH:(h + 1) * NH] = res.results[c]["out"].astype(np.float32)
    return out.reshape(x.shape).astype(x.dtype)
